# revision 7
# baseline (speedup 1.0000x reference)
"""CapsLayer kernel for 8 Trainium2 NeuronCores.

Math: the reference's routing loop is dead (softmax over a size-1 axis is
identically 1), so the output is
    s[b, j, l] = sum_{i,k} W[i, j, l, k] * inputs[b, i, k]
    vj = squash(s, axis=l)   ->  [B, 1, NUM_CAPS, DIM_CAPS]

Sharding: W is split over NUM_CAPS j (4 caps per core, 16.8 MB of W each);
inputs (4 MB) are replicated.  Each core computes its [B, 4*32] slice of s
with a fully on-core contraction (no collectives), applies squash on-device,
and the host concatenates the 8 slices along j.

Layout: the host packs, per core, row i = [inputs^T[i] (IN_DIM*B floats) |
W[i, j_shard] (NJL*IN_DIM floats)] into one [IN_CAPS, XROW+WROW] array.
Each 128-row i-tile is then ONE contiguous DMA (a single HWDGE semaphore —
fp32 matmuls can carry only one sync wait through walrus codegen).  The
contraction runs as IN_DIM accumulation chains: for each k,
lhsT = x part [i, b] (stride-B slice) and rhs = W part [i, (j,l)] read with a
stride-IN_DIM free-dim access pattern, so W needs no transpose anywhere.
"""

import numpy as np

B = 32
IN_CAPS = 2048
IN_DIM = 16
NUM_CAPS = 32
DIM_CAPS = 32
NCORES = 8
JPC = NUM_CAPS // NCORES          # output capsules per core
NJL = JPC * DIM_CAPS              # per-core output columns (j, l)
P = 128                           # partition tile along i
ITILES = IN_CAPS // P
XROW = IN_DIM * B                 # 512 floats of packed x per row
WROW = NJL * IN_DIM               # 2048 floats of packed W per row
ROW = XROW + WROW
EPS = 1e-7

_CACHE = {}


def _build():
    import concourse.bass as bass
    import concourse.tile as tile
    from concourse import mybir

    nc = bass.Bass()
    f32 = mybir.dt.float32
    xw = nc.dram_tensor("xw", [IN_CAPS, ROW], f32, kind="ExternalInput")
    out = nc.dram_tensor("out", [B, NJL], f32, kind="ExternalOutput")

    with tile.TileContext(nc) as tc:
        with (
            tc.tile_pool(name="wp", bufs=16) as wp,
            tc.tile_pool(name="ps", bufs=1, space="PSUM") as ps,
            tc.tile_pool(name="ep", bufs=1) as ep,
        ):
            psum_t = ps.tile([B, NJL], f32)
            for t in range(ITILES):
                xwt = wp.tile([P, ROW], f32)
                nc.sync.dma_start(out=xwt, in_=xw[t * P:(t + 1) * P, :])
                xt_v = xwt[:, :XROW].rearrange("p (k b) -> p k b", b=B)
                wt_v = xwt[:, XROW:].rearrange("p (n k) -> p n k", k=IN_DIM)
                for k in range(IN_DIM):
                    nc.tensor.matmul(
                        psum_t,
                        xt_v[:, k, :],      # lhsT [128, B]
                        wt_v[:, :, k],      # rhs  [128, NJL], stride-16 free AP
                        start=(t == 0 and k == 0),
                        stop=(t == ITILES - 1 and k == IN_DIM - 1),
                    )

            # squash over each group of DIM_CAPS columns.
            # Engine choreography keeps every instruction at <=1 sem wait
            # (this walrus build rejects 2+): the ACT pair (sqrt, +1) is
            # produced back-to-back so the DVE multiply needs only one ACT
            # wait, and the store goes out via SWDGE so it doesn't inherit a
            # same-lane HWDGE ordering wait.
            epst = ep.tile([B, 1], f32)
            nc.vector.memset(epst, EPS)
            s_sb = ep.tile([B, NJL], f32)
            nc.scalar.copy(out=s_sb, in_=psum_t)
            sq = ep.tile([B, NJL], f32)
            nc.vector.tensor_mul(sq, s_sb, s_sb)
            ss = ep.tile([B, JPC], f32)
            for g in range(JPC):
                nc.vector.reduce_sum(
                    out=ss[:, g:g + 1],
                    in_=sq[:, g * DIM_CAPS:(g + 1) * DIM_CAPS],
                    axis=mybir.AxisListType.X,
                )
            rt = ep.tile([B, JPC], f32)
            nc.scalar.activation(
                out=rt, in_=ss, func=mybir.ActivationFunctionType.Sqrt, bias=epst
            )
            a1 = ep.tile([B, JPC], f32)
            nc.scalar.activation(
                out=a1, in_=ss, func=mybir.ActivationFunctionType.Copy, bias=1.0
            )
            den = ep.tile([B, JPC], f32)
            nc.vector.tensor_mul(den, a1, rt)
            rden = ep.tile([B, JPC], f32)
            nc.vector.reciprocal(out=rden, in_=den)
            f = ep.tile([B, JPC], f32)
            nc.vector.tensor_mul(f, ss, rden)
            vout = ep.tile([B, NJL], f32)
            for g in range(JPC):
                nc.vector.tensor_scalar_mul(
                    out=vout[:, g * DIM_CAPS:(g + 1) * DIM_CAPS],
                    in0=s_sb[:, g * DIM_CAPS:(g + 1) * DIM_CAPS],
                    scalar1=f[:, g:g + 1],
                )
            nc.gpsimd.dma_start(out=out[:, :], in_=vout)

    # This walrus build rejects any instruction carrying 2+ sem waits.  The
    # only offender left is Tile's auto-generated kernel-tail Drain, whose
    # 12 waits are causally implied by the final store's semaphore: the
    # SWDGE store waits on DVE, DVE waited on ACT, ACT waited on PE, and PE
    # waited on every HWDGE load lane.  Keep only the store's wait.
    for blk in nc.m.functions[0].blocks:
        for inst in blk.instructions:
            if type(inst).__name__ == "InstDrain":
                si = inst.sync_info
                if si is not None and len(si.on_wait) >= 2:
                    keep = [w for w in si.on_wait if w.ant_name.startswith("DMASW")]
                    assert keep, [w.ant_name for w in si.on_wait]
                    inst.sync_info = type(si)(
                        on_wait=keep, on_update=list(si.on_update)
                    )
    return nc


def _in_maps(inputs, W):
    x_t = np.transpose(inputs, (1, 2, 0)).reshape(IN_CAPS, XROW)  # [i, (k,b)]
    maps = []
    for c in range(NCORES):
        xw = np.empty((IN_CAPS, ROW), dtype=np.float32)
        xw[:, :XROW] = x_t
        xw[:, XROW:] = W[:, c * JPC:(c + 1) * JPC].reshape(IN_CAPS, WROW)
        maps.append({"xw": xw})
    return maps


def kernel(inputs, W):
    from concourse.bass_utils import run_bass_kernel_spmd

    if "nc" not in _CACHE:
        _CACHE["nc"] = _build()
    res = run_bass_kernel_spmd(_CACHE["nc"], _in_maps(inputs, W), list(range(NCORES)))
    return np.concatenate(
        [res.results[c]["out"].reshape(B, 1, JPC, DIM_CAPS) for c in range(NCORES)],
        axis=2,
    )


# revision 10
# speedup vs baseline: 1.0008x; 1.0008x over previous
"""CapsLayer kernel v3: j-sharded, 4-way column-tiled fp32 contraction.

Math: the reference's routing loop is dead (softmax over a size-1 axis is
identically 1), so the output is
    s[b, j, l] = sum_{i,k} W[i, j, l, k] * inputs[b, i, k]
    vj = squash(s, axis=l)  ->  [B, 1, NUM_CAPS, DIM_CAPS]

Sharding: W splits over NUM_CAPS j (4 capsules / 128 output columns per
core, 16.8 MB of W each); inputs (4 MB) are replicated.  Everything stays
on-core — no collectives (an 8-core ncfw ReduceScatter measures ~42 us of
fixed latency, far more than the 4 MB of duplicated input traffic costs).

PE: the contraction runs as 16 accumulation chains (one per k), assigned
round-robin to the four 32-column PE array groups via tile_position, so
four fp32 matmul streams are in flight concurrently and the per-
instruction overhead + fp32 double-pass cost is hidden.  Chain g
accumulates into PSUM partitions [32g, 32g+32).  A final 128x32 identity-
block matmul (E packed into tile 0's rows) folds the four partial chains
into s[b, n], and squash runs on [B=32, 128].

Raw Bass: this walrus build rejects instructions carrying 2+ sem waits, so
all sync is standalone wait_ge ops.  DVE/ACT same-engine RAW needs explicit
semaphores (the pipelines do not interlock through SBUF).
"""

from contextlib import ExitStack

import numpy as np

B = 32
IN_CAPS = 2048
IN_DIM = 16
NUM_CAPS = 32
DIM_CAPS = 32
NCORES = 8
JPC = NUM_CAPS // NCORES          # 4 capsules per core
NJL = JPC * DIM_CAPS              # 128 output columns per core
P = 128
NTILES = IN_CAPS // P             # 16
XROW = IN_DIM * B                 # 512 packed x floats per row (k, b)
WROW = NJL * IN_DIM               # 2048 packed w floats per row (j, l, k)
EROW = B                          # 32 identity-block floats per row
ROW = XROW + WROW + EROW          # 2592
NG = 4                            # PE column groups
EPS = 1e-7

_CACHE = {}


def _build():
    import concourse.bass as bass
    from concourse import mybir

    f32 = mybir.dt.float32
    nc = bass.Bass()
    xw = nc.declare_dram_parameter("xw", [IN_CAPS, ROW], f32, isOutput=False)
    out = nc.declare_dram_parameter("out", [B, NJL], f32, isOutput=True)

    with ExitStack() as ctx:
        xw_sb = ctx.enter_context(nc.sbuf_tensor([P, NTILES * ROW], f32))
        p4_sb = ctx.enter_context(nc.sbuf_tensor([P, NJL], f32))
        sv = ctx.enter_context(nc.sbuf_tensor([B, NJL], f32))
        sq = ctx.enter_context(nc.sbuf_tensor([B, NJL], f32))
        ss = ctx.enter_context(nc.sbuf_tensor([B, JPC], f32))
        rt = ctx.enter_context(nc.sbuf_tensor([B, JPC], f32))
        a1 = ctx.enter_context(nc.sbuf_tensor([B, JPC], f32))
        den = ctx.enter_context(nc.sbuf_tensor([B, JPC], f32))
        rden = ctx.enter_context(nc.sbuf_tensor([B, JPC], f32))
        fsc = ctx.enter_context(nc.sbuf_tensor([B, JPC], f32))
        epst = ctx.enter_context(nc.sbuf_tensor([B, 1], f32))
        warm = ctx.enter_context(nc.sbuf_tensor([B, 1], f32))
        vout = ctx.enter_context(nc.sbuf_tensor([B, NJL], f32))
        ps4 = ctx.enter_context(nc.psum_tensor([P, NJL], f32))
        pss = ctx.enter_context(nc.psum_tensor([B, NJL], f32))

        tsem = [ctx.enter_context(nc.semaphore(f"t{t}")) for t in range(NTILES)]
        pe_sem = ctx.enter_context(nc.semaphore("pe"))
        act_sem = ctx.enter_context(nc.semaphore("act"))
        dve_sem = ctx.enter_context(nc.semaphore("dve"))
        odma = ctx.enter_context(nc.semaphore("odma"))
        block = ctx.enter_context(nc.Block())

        @block.sync
        def _(sync):
            for t in range(NTILES):
                sync.dma_start(
                    out=xw_sb[:, t * ROW:(t + 1) * ROW],
                    in_=xw[t * P:(t + 1) * P, :],
                ).then_inc(tsem[t], 16)
            sync.wait_ge(dve_sem, 7)
            sync.dma_start(out=out[:, :], in_=vout[:, :]).then_inc(odma, 16)
            sync.wait_ge(odma, 16)

        @block.tensor
        def _(tensor):
            for t in range(NTILES):
                base = t * ROW
                tensor.wait_ge(tsem[t], 16)
                wview = xw_sb[:, base + XROW:base + XROW + WROW].rearrange(
                    "p (n k) -> p n k", k=IN_DIM
                )
                for k in range(IN_DIM):
                    g = k % NG
                    mm = nc.tensor.matmul(
                        ps4[32 * g:32 * (g + 1), :],
                        xw_sb[:, base + k * B:base + (k + 1) * B],
                        wview[:, :, k],
                        start=(t == 0 and k < NG),
                        stop=(t == NTILES - 1 and k >= IN_DIM - NG),
                        tile_position=(0, 32 * g),
                    )
            mm.then_inc(pe_sem, 1)
            # merge the 4 partial chains: s[b, n] = sum_g p4[32g+b, n]
            tensor.wait_ge(dve_sem, 1)
            nc.tensor.matmul(
                pss[:, :],
                xw_sb[:, XROW + WROW:ROW],       # E block from tile 0
                p4_sb[:, :],
                start=True,
                stop=True,
            ).then_inc(pe_sem, 1)

        @block.vector
        def _(vector):
            nc.vector.memset(epst[:, :], EPS)
            vector.wait_ge(pe_sem, 1)
            nc.vector.tensor_copy(p4_sb[:, :], ps4[:, :]).then_inc(dve_sem, 1)
            # squash: sq = sv^2, ss[g] = sum over each DIM_CAPS group
            vector.wait_ge(act_sem, 1)
            nc.vector.tensor_mul(sq[:, :], sv[:, :], sv[:, :]).then_inc(dve_sem, 1)
            vector.wait_ge(dve_sem, 2)
            for g in range(JPC):
                red = nc.vector.reduce_sum(
                    out=ss[:, g:g + 1],
                    in_=sq[:, g * DIM_CAPS:(g + 1) * DIM_CAPS],
                    axis=mybir.AxisListType.X,
                )
            red.then_inc(dve_sem, 1)
            vector.wait_ge(act_sem, 2)
            nc.vector.tensor_mul(den[:, :], a1[:, :], rt[:, :]).then_inc(dve_sem, 1)
            vector.wait_ge(dve_sem, 4)
            nc.vector.reciprocal(out=rden[:, :], in_=den[:, :]).then_inc(dve_sem, 1)
            vector.wait_ge(dve_sem, 5)
            nc.vector.tensor_mul(fsc[:, :], ss[:, :], rden[:, :]).then_inc(
                dve_sem, 1
            )
            vector.wait_ge(dve_sem, 6)
            for g in range(JPC):
                tsm = nc.vector.tensor_scalar_mul(
                    out=vout[:, g * DIM_CAPS:(g + 1) * DIM_CAPS],
                    in0=sv[:, g * DIM_CAPS:(g + 1) * DIM_CAPS],
                    scalar1=fsc[:, g:g + 1],
                )
            tsm.then_inc(dve_sem, 1)

        @block.scalar
        def _(scalar):
            # dummy Sqrt at t=0 pulls the ~1.3us ACT table load off the
            # epilogue critical path (operands are a scratch tile nobody
            # else touches; the value is unused)
            nc.scalar.activation(
                out=warm[:, :], in_=warm[:, :],
                func=mybir.ActivationFunctionType.Sqrt, bias=warm[:, :],
            )
            scalar.wait_ge(pe_sem, 2)
            nc.scalar.copy(out=sv[:, :], in_=pss[:, :]).then_inc(act_sem, 1)
            scalar.wait_ge(dve_sem, 3)
            nc.scalar.activation(
                out=rt[:, :], in_=ss[:, :],
                func=mybir.ActivationFunctionType.Sqrt, bias=epst[:, :],
            )
            nc.scalar.activation(
                out=a1[:, :], in_=ss[:, :],
                func=mybir.ActivationFunctionType.Copy, bias=1.0,
            ).then_inc(act_sem, 1)

    return nc


def _in_maps(inputs, W):
    x_t = np.transpose(inputs, (1, 2, 0)).reshape(IN_CAPS, XROW)  # [i, (k, b)]
    erow = np.zeros((IN_CAPS, B), dtype=np.float32)
    erow[np.arange(IN_CAPS), np.arange(IN_CAPS) % B] = 1.0       # E[p%32 == b]
    maps = []
    for c in range(NCORES):
        xwc = np.empty((IN_CAPS, ROW), dtype=np.float32)
        xwc[:, :XROW] = x_t
        xwc[:, XROW:XROW + WROW] = W[:, c * JPC:(c + 1) * JPC].reshape(
            IN_CAPS, WROW
        )
        xwc[:, XROW + WROW:] = erow
        maps.append({"xw": xwc})
    return maps


def kernel(inputs, W):
    from concourse.bass_utils import run_bass_kernel_spmd

    if "nc" not in _CACHE:
        _CACHE["nc"] = _build()
    res = run_bass_kernel_spmd(_CACHE["nc"], _in_maps(inputs, W), list(range(NCORES)))
    return np.concatenate(
        [res.results[c]["out"].reshape(B, 1, JPC, DIM_CAPS) for c in range(NCORES)],
        axis=2,
    )


# revision 11
# speedup vs baseline: 1.0137x; 1.0130x over previous
"""CapsLayer kernel v3: j-sharded, 4-way column-tiled fp32 contraction.

Math: the reference's routing loop is dead (softmax over a size-1 axis is
identically 1), so the output is
    s[b, j, l] = sum_{i,k} W[i, j, l, k] * inputs[b, i, k]
    vj = squash(s, axis=l)  ->  [B, 1, NUM_CAPS, DIM_CAPS]

Sharding: W splits over NUM_CAPS j (4 capsules / 128 output columns per
core, 16.8 MB of W each); inputs (4 MB) are replicated.  Everything stays
on-core — no collectives (an 8-core ncfw ReduceScatter measures ~42 us of
fixed latency, far more than the 4 MB of duplicated input traffic costs).

PE: the contraction runs as 16 accumulation chains (one per k), assigned
round-robin to the four 32-column PE array groups via tile_position, so
four fp32 matmul streams are in flight concurrently and the per-
instruction overhead + fp32 double-pass cost is hidden.  Chain g
accumulates into PSUM partitions [32g, 32g+32).  A final 128x32 identity-
block matmul (E packed into tile 0's rows) folds the four partial chains
into s[b, n], and squash runs on [B=32, 128].

Raw Bass: this walrus build rejects instructions carrying 2+ sem waits, so
all sync is standalone wait_ge ops.  DVE/ACT same-engine RAW needs explicit
semaphores (the pipelines do not interlock through SBUF).
"""

from contextlib import ExitStack

import numpy as np

B = 32
IN_CAPS = 2048
IN_DIM = 16
NUM_CAPS = 32
DIM_CAPS = 32
NCORES = 8
JPC = NUM_CAPS // NCORES          # 4 capsules per core
NJL = JPC * DIM_CAPS              # 128 output columns per core
P = 128
NTILES = IN_CAPS // P             # 16
XROW = IN_DIM * B                 # 512 packed x floats per row (k, b)
WROW = NJL * IN_DIM               # 2048 packed w floats per row (j, l, k)
EROW = B                          # 32 identity-block floats per row
ROW = XROW + WROW + EROW          # 2592
NG = 4                            # PE column groups
EPS = 1e-7

_CACHE = {}


def _build():
    import concourse.bass as bass
    from concourse import mybir

    f32 = mybir.dt.float32
    nc = bass.Bass()
    xw = nc.declare_dram_parameter("xw", [IN_CAPS, ROW], f32, isOutput=False)
    out = nc.declare_dram_parameter("out", [B, NJL], f32, isOutput=True)

    with ExitStack() as ctx:
        xw_sb = ctx.enter_context(nc.sbuf_tensor([P, NTILES * ROW], f32))
        p4_sb = ctx.enter_context(nc.sbuf_tensor([P, NJL], f32))
        sv = ctx.enter_context(nc.sbuf_tensor([B, NJL], f32))
        sq = ctx.enter_context(nc.sbuf_tensor([B, NJL], f32))
        ss = ctx.enter_context(nc.sbuf_tensor([B, JPC], f32))
        rt = ctx.enter_context(nc.sbuf_tensor([B, JPC], f32))
        a1 = ctx.enter_context(nc.sbuf_tensor([B, JPC], f32))
        den = ctx.enter_context(nc.sbuf_tensor([B, JPC], f32))
        rden = ctx.enter_context(nc.sbuf_tensor([B, JPC], f32))
        fsc = ctx.enter_context(nc.sbuf_tensor([B, JPC], f32))
        epst = ctx.enter_context(nc.sbuf_tensor([B, 1], f32))
        warm = ctx.enter_context(nc.sbuf_tensor([B, 1], f32))
        vout = ctx.enter_context(nc.sbuf_tensor([B, NJL], f32))
        ps4 = ctx.enter_context(nc.psum_tensor([P, NJL], f32))
        pss = ctx.enter_context(nc.psum_tensor([B, NJL], f32))

        tsem = [ctx.enter_context(nc.semaphore(f"t{t}")) for t in range(NTILES)]
        pe_sem = ctx.enter_context(nc.semaphore("pe"))
        act_sem = ctx.enter_context(nc.semaphore("act"))
        dve_sem = ctx.enter_context(nc.semaphore("dve"))
        odma = ctx.enter_context(nc.semaphore("odma"))
        block = ctx.enter_context(nc.Block())

        @block.sync
        def _(sync):
            for t in range(NTILES):
                sync.dma_start(
                    out=xw_sb[:, t * ROW:(t + 1) * ROW],
                    in_=xw[t * P:(t + 1) * P, :],
                ).then_inc(tsem[t], 16)
            sync.wait_ge(dve_sem, 7)
            sync.dma_start(out=out[:, :], in_=vout[:, :]).then_inc(odma, 16)
            sync.wait_ge(odma, 16)

        @block.tensor
        def _(tensor):
            for t in range(NTILES):
                base = t * ROW
                tensor.wait_ge(tsem[t], 16)
                wview = xw_sb[:, base + XROW:base + XROW + WROW].rearrange(
                    "p (n k) -> p n k", k=IN_DIM
                )
                for k in range(IN_DIM):
                    g = k % NG
                    mm = nc.tensor.matmul(
                        ps4[32 * g:32 * (g + 1), :],
                        xw_sb[:, base + k * B:base + (k + 1) * B],
                        wview[:, :, k],
                        start=(t == 0 and k < NG),
                        stop=(t == NTILES - 1 and k >= IN_DIM - NG),
                        tile_position=(0, 32 * g),
                    )
            mm.then_inc(pe_sem, 1)
            # merge the 4 partial chains: s[b, n] = sum_g p4[32g+b, n]
            tensor.wait_ge(dve_sem, 1)
            nc.tensor.matmul(
                pss[:, :],
                xw_sb[:, XROW + WROW:ROW],       # E block from tile 0
                p4_sb[:, :],
                start=True,
                stop=True,
            ).then_inc(pe_sem, 1)

        @block.vector
        def _(vector):
            nc.vector.memset(epst[:, :], EPS)
            vector.wait_ge(pe_sem, 1)
            nc.vector.tensor_copy(p4_sb[:, :], ps4[:, :]).then_inc(dve_sem, 1)
            # squash: sq = sv^2, ss[g] = sum over each DIM_CAPS group
            vector.wait_ge(act_sem, 1)
            nc.vector.tensor_mul(sq[:, :], sv[:, :], sv[:, :]).then_inc(dve_sem, 1)
            vector.wait_ge(dve_sem, 2)
            for g in range(JPC):
                red = nc.vector.reduce_sum(
                    out=ss[:, g:g + 1],
                    in_=sq[:, g * DIM_CAPS:(g + 1) * DIM_CAPS],
                    axis=mybir.AxisListType.X,
                )
            red.then_inc(dve_sem, 1)
            vector.wait_ge(act_sem, 2)
            nc.vector.tensor_mul(den[:, :], a1[:, :], rt[:, :]).then_inc(dve_sem, 1)
            vector.wait_ge(dve_sem, 4)
            nc.vector.reciprocal(out=rden[:, :], in_=den[:, :]).then_inc(dve_sem, 1)
            vector.wait_ge(dve_sem, 5)
            nc.vector.tensor_mul(fsc[:, :], ss[:, :], rden[:, :]).then_inc(
                dve_sem, 1
            )
            vector.wait_ge(dve_sem, 6)
            for g in range(JPC):
                tsm = nc.vector.tensor_scalar_mul(
                    out=vout[:, g * DIM_CAPS:(g + 1) * DIM_CAPS],
                    in0=sv[:, g * DIM_CAPS:(g + 1) * DIM_CAPS],
                    scalar1=fsc[:, g:g + 1],
                )
            tsm.then_inc(dve_sem, 1)

        @block.scalar
        def _(scalar):
            # dummy Sqrt at t=0 pulls the ~1.3us ACT table load off the
            # epilogue critical path (operands are a scratch tile nobody
            # else touches; the value is unused)
            nc.scalar.activation(
                out=warm[:, :], in_=warm[:, :],
                func=mybir.ActivationFunctionType.Sqrt, bias=warm[:, :],
            )
            scalar.wait_ge(pe_sem, 2)
            nc.scalar.copy(out=sv[:, :], in_=pss[:, :]).then_inc(act_sem, 1)
            scalar.wait_ge(dve_sem, 3)
            nc.scalar.activation(
                out=rt[:, :], in_=ss[:, :],
                func=mybir.ActivationFunctionType.Sqrt, bias=epst[:, :],
            )
            nc.scalar.activation(
                out=a1[:, :], in_=ss[:, :],
                func=mybir.ActivationFunctionType.Copy, bias=1.0,
            ).then_inc(act_sem, 1)

    return nc


def _in_maps(inputs, W):
    x_t = np.transpose(inputs, (1, 2, 0)).reshape(IN_CAPS, XROW)  # [i, (k, b)]
    erow = np.zeros((IN_CAPS, B), dtype=np.float32)
    erow[np.arange(IN_CAPS), np.arange(IN_CAPS) % B] = 1.0       # E[p%32 == b]
    maps = []
    for c in range(NCORES):
        xwc = np.empty((IN_CAPS, ROW), dtype=np.float32)
        xwc[:, :XROW] = x_t
        xwc[:, XROW:XROW + WROW] = W[:, c * JPC:(c + 1) * JPC].reshape(
            IN_CAPS, WROW
        )
        xwc[:, XROW + WROW:] = erow
        maps.append({"xw": xwc})
    return maps


def kernel(inputs, W):
    from concourse.bass_utils import run_bass_kernel_spmd

    inputs = np.asarray(inputs, dtype=np.float32)
    W = np.asarray(W, dtype=np.float32)
    if "nc" not in _CACHE:
        _CACHE["nc"] = _build()
    res = run_bass_kernel_spmd(_CACHE["nc"], _in_maps(inputs, W), list(range(NCORES)))
    return np.concatenate(
        [res.results[c]["out"].reshape(B, 1, JPC, DIM_CAPS) for c in range(NCORES)],
        axis=2,
    )


# revision 12
# speedup vs baseline: 1.0990x; 1.0842x over previous
"""CapsLayer kernel v3: j-sharded, 4-way column-tiled fp32 contraction.

Math: the reference's routing loop is dead (softmax over a size-1 axis is
identically 1), so the output is
    s[b, j, l] = sum_{i,k} W[i, j, l, k] * inputs[b, i, k]
    vj = squash(s, axis=l)  ->  [B, 1, NUM_CAPS, DIM_CAPS]

Sharding: W splits over NUM_CAPS j (4 capsules / 128 output columns per
core, 16.8 MB of W each); inputs (4 MB) are replicated.  Everything stays
on-core — no collectives (an 8-core ncfw ReduceScatter measures ~42 us of
fixed latency, far more than the 4 MB of duplicated input traffic costs).

PE: the contraction runs as 16 accumulation chains (one per k), assigned
round-robin to the four 32-column PE array groups via tile_position, so
four fp32 matmul streams are in flight concurrently and the per-
instruction overhead + fp32 double-pass cost is hidden.  Chain g
accumulates into PSUM partitions [32g, 32g+32).  A final 128x32 identity-
block matmul (E packed into tile 0's rows) folds the four partial chains
into s[b, n], and squash runs on [B=32, 128].

Raw Bass: this walrus build rejects instructions carrying 2+ sem waits, so
all sync is standalone wait_ge ops.  DVE/ACT same-engine RAW needs explicit
semaphores (the pipelines do not interlock through SBUF).
"""

from contextlib import ExitStack

import numpy as np

B = 32
IN_CAPS = 2048
IN_DIM = 16
NUM_CAPS = 32
DIM_CAPS = 32
NCORES = 8
JPC = NUM_CAPS // NCORES          # 4 capsules per core
NJL = JPC * DIM_CAPS              # 128 output columns per core
P = 128
NTILES = IN_CAPS // P             # 16
XROW = IN_DIM * B                 # 512 packed x floats per row (k, b)
WROW = NJL * IN_DIM               # 2048 packed w floats per row (j, l, k)
EROW = B                          # 32 identity-block floats per row
ROW = XROW + WROW + EROW          # 2592
NG = 4                            # PE column groups
EPS = 1e-7

_CACHE = {}


def _build():
    import concourse.bass as bass
    from concourse import mybir

    f32 = mybir.dt.float32
    nc = bass.Bass()
    xw = nc.declare_dram_parameter("xw", [IN_CAPS, ROW], f32, isOutput=False)
    out = nc.declare_dram_parameter("out", [B, NJL], f32, isOutput=True)

    with ExitStack() as ctx:
        xw_sb = ctx.enter_context(nc.sbuf_tensor([P, NTILES * ROW], f32))
        p4_sb = ctx.enter_context(nc.sbuf_tensor([P, NJL], f32))
        sv = ctx.enter_context(nc.sbuf_tensor([B, NJL], f32))
        sq = ctx.enter_context(nc.sbuf_tensor([B, NJL], f32))
        ss = ctx.enter_context(nc.sbuf_tensor([B, JPC], f32))
        rt = ctx.enter_context(nc.sbuf_tensor([B, JPC], f32))
        a1 = ctx.enter_context(nc.sbuf_tensor([B, JPC], f32))
        den = ctx.enter_context(nc.sbuf_tensor([B, JPC], f32))
        rden = ctx.enter_context(nc.sbuf_tensor([B, JPC], f32))
        fsc = ctx.enter_context(nc.sbuf_tensor([B, JPC], f32))
        epst = ctx.enter_context(nc.sbuf_tensor([B, 1], f32))
        warm = ctx.enter_context(nc.sbuf_tensor([B, 1], f32))
        vout = ctx.enter_context(nc.sbuf_tensor([B, NJL], f32))
        ps4 = ctx.enter_context(nc.psum_tensor([P, NJL], f32))
        pss = ctx.enter_context(nc.psum_tensor([B, NJL], f32))

        tsem = [ctx.enter_context(nc.semaphore(f"t{t}")) for t in range(NTILES)]
        pe_sem = ctx.enter_context(nc.semaphore("pe"))
        act_sem = ctx.enter_context(nc.semaphore("act"))
        dve_sem = ctx.enter_context(nc.semaphore("dve"))
        odma = ctx.enter_context(nc.semaphore("odma"))
        block = ctx.enter_context(nc.Block())

        @block.sync
        def _(sync):
            for t in range(NTILES):
                sync.dma_start(
                    out=xw_sb[:, t * ROW:(t + 1) * ROW],
                    in_=xw[t * P:(t + 1) * P, :],
                ).then_inc(tsem[t], 16)
            sync.wait_ge(dve_sem, 7)
            sync.dma_start(out=out[:, :], in_=vout[:, :]).then_inc(odma, 16)
            sync.wait_ge(odma, 16)

        @block.tensor
        def _(tensor):
            for t in range(NTILES):
                base = t * ROW
                tensor.wait_ge(tsem[t], 16)
                wview = xw_sb[:, base + XROW:base + XROW + WROW].rearrange(
                    "p (n k) -> p n k", k=IN_DIM
                )
                for k in range(IN_DIM):
                    g = k % NG
                    mm = nc.tensor.matmul(
                        ps4[32 * g:32 * (g + 1), :],
                        xw_sb[:, base + k * B:base + (k + 1) * B],
                        wview[:, :, k],
                        start=(t == 0 and k < NG),
                        stop=(t == NTILES - 1 and k >= IN_DIM - NG),
                        tile_position=(0, 32 * g),
                    )
            mm.then_inc(pe_sem, 1)
            # merge the 4 partial chains: s[b, n] = sum_g p4[32g+b, n]
            tensor.wait_ge(dve_sem, 1)
            nc.tensor.matmul(
                pss[:, :],
                xw_sb[:, XROW + WROW:ROW],       # E block from tile 0
                p4_sb[:, :],
                start=True,
                stop=True,
            ).then_inc(pe_sem, 1)

        @block.vector
        def _(vector):
            nc.vector.memset(epst[:, :], EPS)
            vector.wait_ge(pe_sem, 1)
            nc.vector.tensor_copy(p4_sb[:, :], ps4[:, :]).then_inc(dve_sem, 1)
            # squash: sq = sv^2, ss[g] = sum over each DIM_CAPS group
            vector.wait_ge(act_sem, 1)
            nc.vector.tensor_mul(sq[:, :], sv[:, :], sv[:, :]).then_inc(dve_sem, 1)
            vector.wait_ge(dve_sem, 2)
            red = nc.vector.reduce_sum(
                out=ss[:, :],
                in_=sq[:, :].rearrange("p (g d) -> p g d", g=JPC),
                axis=mybir.AxisListType.X,
            )
            red.then_inc(dve_sem, 1)
            vector.wait_ge(act_sem, 2)
            nc.vector.tensor_mul(den[:, :], a1[:, :], rt[:, :]).then_inc(dve_sem, 1)
            vector.wait_ge(dve_sem, 4)
            nc.vector.reciprocal(out=rden[:, :], in_=den[:, :]).then_inc(dve_sem, 1)
            vector.wait_ge(dve_sem, 5)
            nc.vector.tensor_mul(fsc[:, :], ss[:, :], rden[:, :]).then_inc(
                dve_sem, 1
            )
            vector.wait_ge(dve_sem, 6)
            for g in range(JPC):
                tsm = nc.vector.tensor_scalar_mul(
                    out=vout[:, g * DIM_CAPS:(g + 1) * DIM_CAPS],
                    in0=sv[:, g * DIM_CAPS:(g + 1) * DIM_CAPS],
                    scalar1=fsc[:, g:g + 1],
                )
            tsm.then_inc(dve_sem, 1)

        @block.scalar
        def _(scalar):
            # dummy Sqrt at t=0 pulls the ~1.3us ACT table load off the
            # epilogue critical path (operands are a scratch tile nobody
            # else touches; the value is unused)
            nc.scalar.activation(
                out=warm[:, :], in_=warm[:, :],
                func=mybir.ActivationFunctionType.Sqrt, bias=warm[:, :],
            )
            scalar.wait_ge(pe_sem, 2)
            nc.scalar.copy(out=sv[:, :], in_=pss[:, :]).then_inc(act_sem, 1)
            scalar.wait_ge(dve_sem, 3)
            nc.scalar.activation(
                out=rt[:, :], in_=ss[:, :],
                func=mybir.ActivationFunctionType.Sqrt, bias=epst[:, :],
            )
            nc.scalar.activation(
                out=a1[:, :], in_=ss[:, :],
                func=mybir.ActivationFunctionType.Copy, bias=1.0,
            ).then_inc(act_sem, 1)

    return nc


def _in_maps(inputs, W):
    x_t = np.transpose(inputs, (1, 2, 0)).reshape(IN_CAPS, XROW)  # [i, (k, b)]
    erow = np.zeros((IN_CAPS, B), dtype=np.float32)
    erow[np.arange(IN_CAPS), np.arange(IN_CAPS) % B] = 1.0       # E[p%32 == b]
    maps = []
    for c in range(NCORES):
        xwc = np.empty((IN_CAPS, ROW), dtype=np.float32)
        xwc[:, :XROW] = x_t
        xwc[:, XROW:XROW + WROW] = W[:, c * JPC:(c + 1) * JPC].reshape(
            IN_CAPS, WROW
        )
        xwc[:, XROW + WROW:] = erow
        maps.append({"xw": xwc})
    return maps


def kernel(inputs, W):
    from concourse.bass_utils import run_bass_kernel_spmd

    inputs = np.asarray(inputs, dtype=np.float32)
    W = np.asarray(W, dtype=np.float32)
    if "nc" not in _CACHE:
        _CACHE["nc"] = _build()
    res = run_bass_kernel_spmd(_CACHE["nc"], _in_maps(inputs, W), list(range(NCORES)))
    return np.concatenate(
        [res.results[c]["out"].reshape(B, 1, JPC, DIM_CAPS) for c in range(NCORES)],
        axis=2,
    )
